# revision 38
# baseline (speedup 1.0000x reference)
"""CAM (channel attention) kernel for Trainium2, SPMD over 8 NeuronCores.

Computation per batch b (reference semantics):
    v      = x[b].reshape(C, N)                      # C=512, N=4096
    energy = v @ v.T                                 # [C, C] Gram over channels
    att    = softmax(max_j(energy) - energy, axis=-1)
           = exp(min_j(energy) - energy) / sum_j(...)   # algebraically identical
    out    = gamma * (att @ v) + x[b]

Distribution: pure data parallel over batch. B=16 -> 2 batches per core.

The kernel sits on the DMA roofline: 16.8MB in + 16.8MB out per core at
~360-400 GB/s ≈ 90us. Matmul work runs in fp8e4 with the DoubleRow perf
mode (256-deep contraction per instruction, 2x rate) so the PE fits
under that roofline; softmax statistics (row min / exp / sum) stay in
fp32 and the residual path is exact fp32.

Per-core pipeline (per batch):
  - x loaded f32 in 6 chunks on the sync HWDGE ring (first chunk 128px
    so the PE starts ~7us in); stores ride the same FIFO ring, which
    auto-prioritizes all loads ahead of all stores
  - each chunk is rounded to fp8 into v8[tp] [128, 2, N] paired layout
    (dim1 = two 128-channel halves, the DoubleRow contraction pairs);
    the converting casts run on whichever engine is idle during that
    batch's Gram phase (DVE for batch 0; ACT/DVE for batch 1, whose
    Gram phase overlaps batch 0's DVE fuse stream)
  - PE transposes v8 128x128 blocks (fp8, step-2 PSUM writes) into
    u8[kp] [128, 2, C] k-pair tiles; ACT evacuates PSUM->SBUF
  - energy: 16 DoubleRow matmuls per m-tile accumulate the Gram in 4
    f32 PSUM banks, upper-triangle blocks only + 5 transpose fills
  - row-softmax: DVE row-min, ACT exp(bias=min, scale=-1) with fused
    f32 row-sum, fp8 att out; DVE reciprocal; gr = gamma / sum
  - att^T via fp8 PE transposes -> at8[tp] [128, 2, C] paired layout
  - out m-tiles: po = sum_tp at8[tp].T @DR v8[tp] per 512-px chunk
  - evacuation fuses scale+residual IN PLACE over the f32 x tiles (one
    DVE op) and streams each piece out; batch b+1's Gram phase is
    interleaved with batch b's out phase so both engines stay fed
"""

import numpy as np

import concourse.bass as bass
import concourse.bacc as bacc
import concourse.tile as tile
from concourse import mybir
from concourse.bass_utils import run_bass_kernel_spmd
from concourse.masks import make_identity

F32 = mybir.dt.float32
FP8 = mybir.dt.float8e4
DR = mybir.MatmulPerfMode.DoubleRow

B, C, H, W = 16, 512, 64, 64
N = H * W                  # 4096
NCORES = 8
BPC = B // NCORES          # batches per core = 2
CT = C // 128              # 4 channel tiles
KT = N // 128              # 32 contraction tiles for the Gram matrix
KP = KT // 2               # 16 DoubleRow contraction pairs
FT = N // 512              # 8 free-dim chunks for the out matmul
TD = 3                     # transpose software-pipeline depth (k-pairs ahead)
# v is loaded as independent SBUF tiles (start, len in pixels). The first is
# tiny so the PE can start transposing early; boundaries are 256-aligned so
# 256-px k-pair transpose reads never cross tiles (except the 128 head, which
# pairs with the start of chunk 1 -- handled by per-128 transpose reads).
CHUNKS = ((0, 128), (128, 384), (512, 512), (1024, 1024), (2048, 1024), (3072, 1024))


def _chunk_of(n0):
    for lc, (s, ln) in enumerate(CHUNKS):
        if s <= n0 < s + ln:
            return lc, s, ln
    raise AssertionError(n0)


def _segments(n0, w):
    """Split [n0, n0+w) at chunk boundaries -> (chunk idx, start, len)."""
    out = []
    while w > 0:
        lc, s, ln = _chunk_of(n0)
        take = min(w, s + ln - n0)
        out.append((lc, n0, take))
        n0 += take
        w -= take
    return out


def build():
    nc = bacc.Bacc(
        "TRN2",
        target_bir_lowering=False,
        debug=False,
        num_devices=NCORES,
    )
    x_d = nc.dram_tensor("x", [BPC, C, N], F32, kind="ExternalInput")
    g_d = nc.dram_tensor("gamma", [1], F32, kind="ExternalInput")
    o_d = nc.dram_tensor("out", [BPC, C, N], F32, kind="ExternalOutput")
    x_ap, g_ap, o_ap = x_d.ap(), g_d.ap(), o_d.ap()

    with tile.TileContext(nc) as tc:
        with (
            tc.tile_pool(name="const", bufs=1) as const_pool,
            tc.tile_pool(name="vb", bufs=2) as v_pool,
            tc.tile_pool(name="v8", bufs=2) as v8_pool,
            tc.tile_pool(name="u8", bufs=8) as u8_pool,
            tc.tile_pool(name="att", bufs=1) as att_pool,
            tc.tile_pool(name="at8", bufs=2) as at8_pool,
            tc.tile_pool(name="stats", bufs=4) as stats_pool,
            tc.tile_pool(name="stg", bufs=5) as stg_pool,
            tc.tile_pool(name="gr", bufs=2) as gr_pool,
            tc.tile_pool(name="epsum", bufs=1, space="PSUM") as e_pool,
            tc.tile_pool(name="tpsum", bufs=2, space="PSUM") as t_pool,
            tc.tile_pool(name="opsum", bufs=2, space="PSUM") as o_pool,
        ):
            ident = const_pool.tile([128, 128], F32)
            make_identity(nc, ident)
            ident8 = const_pool.tile([128, 128], FP8, name="ident8")
            nc.scalar.copy(ident8, ident)

            gam = const_pool.tile([128, 1], F32)
            nc.gpsimd.dma_start(out=gam, in_=g_ap.to_broadcast((128, 1)))

            # per-batch state carried from phase 1 to phase 2
            state = {}

            def vcol(vt, ci, n0, w):
                # slice [128, w] of channel-tile ci at pixel offset n0 out of
                # the chunked v tiles (w must not cross a chunk boundary)
                lc, s, ln = _chunk_of(n0)
                assert n0 + w <= s + ln, (n0, w)
                return vt[lc][:, ci, n0 - s : n0 - s + w]

            def phase1_gen(b):
                vt = [
                    v_pool.tile([128, CT, ln], F32, tag=f"vb{lc}", name=f"vb{lc}")
                    for lc, (s, ln) in enumerate(CHUNKS)
                ]
                xb = x_ap[b].rearrange("(c p) n -> p c n", p=128)
                for lc, (s, ln) in enumerate(CHUNKS):
                    nc.sync.dma_start(out=vt[lc], in_=xb[:, :, s : s + ln])
                # fp8 paired copies: v8[tp][p, h, n] = fp8(v[(2tp+h)*128+p, n])
                v8 = [
                    v8_pool.tile([128, 2, N], FP8, tag=f"v8_{tp}", name=f"v8_{tp}")
                    for tp in range(2)
                ]

                def convert_chunk(lc):
                    # phase-aware engine split: during batch 0's Gram phase
                    # DVE is idle (fast SBUF->SBUF casts), so it takes all of
                    # b0's conversions and ACT keeps a clean u8-evacuation
                    # stream. During batch 1's Gram phase DVE is busy fusing
                    # b0, so b1's conversions go to ACT (small early chunks)
                    # and Pool/DVE (large chunks, only needed late by the
                    # out-matmuls).
                    s, ln = CHUNKS[lc]
                    for tp in range(2):
                        dst = v8[tp][:, :, s : s + ln]
                        src = vt[lc][:, 2 * tp : 2 * tp + 2, :]
                        # always DVE: idle during b0's Gram phase, and for b1
                        # the eager-emitted casts queue ahead of b0's fuse
                        # groups, keeping ACT purely on the evacuation stream
                        nc.vector.tensor_copy(out=dst, in_=src)

                yield  # [L] loads issued

                e = [
                    e_pool.tile([128, C], F32, tag=f"e{m}", name=f"e{m}")
                    for m in range(CT)
                ]

                def energy_mms(kp, u):
                    # symmetry: only compute j >= 128*min(m,2) (block (3,2)
                    # recomputed to keep the moving dim >= 256)
                    for m in range(CT):
                        j0 = min(m, 2) * 128
                        nc.tensor.matmul(
                            e[m][:, j0:],
                            u[:, :, bass.ts(m, 128)],
                            u[:, :, j0:],
                            start=(kp == 0),
                            stop=(kp == KP - 1),
                            perf_mode=DR,
                        )

                converted = 0
                pending = []
                for kp in range(KP):
                    # convert chunks just ahead of the transposes needing them.
                    # For batch 1 convert everything up front: its DVE casts
                    # then queue ahead of most of batch 0's fuse groups (which
                    # have slack -- b0's stores are ring-bound anyway), so
                    # batch 1's Gram phase isn't paced by the fuse stream.
                    while converted < len(CHUNKS) and (
                        b == 1
                        or CHUNKS[converted][0] < min(kp + TD + 1, KP) * 256
                    ):
                        convert_chunk(converted)
                        converted += 1
                    up = t_pool.tile(
                        [128, 2, C, 2], FP8, tag="upsum", name="upsum"
                    )
                    for h in range(2):
                        k = 2 * kp + h
                        for ci in range(CT):
                            nc.tensor.transpose(
                                up[:, h, bass.ts(ci, 128), 0],
                                v8[ci // 2][:, ci % 2, bass.ts(k, 128)],
                                ident8,
                            )
                    u = u8_pool.tile([128, 2, C], FP8, tag="u", name="u")
                    # batch 0 alternates the PSUM evacuation with the
                    # otherwise-idle DVE so two k-pairs drain in parallel;
                    # batch 1 keeps DVE free for batch 0's fuse stream
                    if b == 0 and kp % 2 == 1:
                        nc.vector.tensor_copy(out=u, in_=up[:, :, :, 0])
                    else:
                        nc.scalar.copy(u, up[:, :, :, 0])
                    pending.append((kp, u))
                    if len(pending) > TD:
                        energy_mms(*pending.pop(0))
                    if kp % 2 == 1:
                        yield  # [K] every 2 k-pairs (8 yields)
                while pending:
                    energy_mms(*pending.pop(0))
                yield  # [E] energy fully emitted

                # fill the skipped lower-triangle blocks: e[m][:,jb] = e[jb][:,m]^T
                # (all 5 staging copies first so ACT streams them while the PE
                # transposes follow, instead of ping-ponging per block)
                tmps = []
                for m, jb in ((1, 0), (2, 0), (2, 1), (3, 0), (3, 1)):
                    tmp = stats_pool.tile(
                        [128, 128], F32, tag="efill", name="efill", bufs=5
                    )
                    nc.scalar.copy(tmp, e[jb][:, bass.ts(m, 128)])
                    tmps.append((m, jb, tmp))
                for m, jb, tmp in tmps:
                    nc.tensor.transpose(e[m][:, bass.ts(jb, 128)], tmp, ident)

                # row softmax (reversed-max form): att = exp(min - e) / sum
                att = []
                gr = []
                for m in range(CT):
                    mn = stats_pool.tile([128, 1], F32, tag="mn", name="mn")
                    nc.vector.tensor_reduce(
                        mn, e[m], axis=mybir.AxisListType.X, op=mybir.AluOpType.min
                    )
                    a = att_pool.tile([128, C], FP8, tag=f"att{m}", name=f"att{m}")
                    s = stats_pool.tile([128, 1], F32, tag="s", name="s")
                    nc.scalar.activation(
                        a,
                        e[m],
                        mybir.ActivationFunctionType.Exp,
                        bias=mn,
                        scale=-1.0,
                        accum_out=s,
                    )
                    r = stats_pool.tile([128, 1], F32, tag="r", name="r")
                    nc.vector.reciprocal(r, s)
                    g = gr_pool.tile([128, 1], F32, tag=f"gr{m}", name=f"gr{m}")
                    nc.vector.tensor_scalar_mul(g, r, gam[:, 0:1])
                    att.append(a)
                    gr.append(g)

                state[b] = (vt, v8, att, gr)

            def phase2_gen(b):
                vt, v8, att, gr = state.pop(b)

                # attT in fp8 paired (DoubleRow) layout:
                # at8[tp][p, h, i] = att[i, (2*tp+h)*128 + p]
                at8 = [
                    at8_pool.tile([128, 2, C], FP8, tag=f"at8_{tp}", name=f"at8_{tp}")
                    for tp in range(2)
                ]
                for tj in range(CT):
                    ap_ps = t_pool.tile(
                        [128, 2, C, 2], FP8, tag="upsum", name="atpsum"
                    )
                    for ti in range(CT):
                        nc.tensor.transpose(
                            ap_ps[:, 0, bass.ts(ti, 128), 0],
                            att[ti][:, bass.ts(tj, 128)],
                            ident8,
                        )
                    nc.scalar.copy(at8[tj // 2][:, tj % 2], ap_ps[:, 0, :, 0])

                for f in range(FT):
                    for ti in range(CT):
                        po = o_pool.tile([128, 512], F32, tag="opsum", name="opsum")
                        for tp in range(2):
                            nc.tensor.matmul(
                                po,
                                at8[tp][:, :, bass.ts(ti, 128)],
                                v8[tp][:, :, bass.ts(f, 512)],
                                start=(tp == 0),
                                stop=(tp == 1),
                                perf_mode=DR,
                            )
                        # final = (po * (gamma/sum_i)) + x, fused IN PLACE over
                        # the x tile, then streamed out. ti==3 takes a 2-op
                        # detour (ACT scale-evacuates PSUM, Pool adds the
                        # residual SBUF-only) so the DVE fuse rate stops
                        # pacing the whole out phase.
                        if ti == 3:
                            tmp = stg_pool.tile([128, 512], F32, tag="p3", name="p3")
                            nc.scalar.activation(
                                tmp,
                                po,
                                mybir.ActivationFunctionType.Copy,
                                scale=gr[ti][:, 0:1],
                            )
                        for lc, s, ln in _segments(f * 512, 512):
                            xs = vcol(vt, ti, s, ln)
                            sl = slice(s - f * 512, s - f * 512 + ln)
                            if ti == 3:
                                nc.gpsimd.tensor_tensor(
                                    out=xs,
                                    in0=tmp[:, sl],
                                    in1=xs,
                                    op=mybir.AluOpType.add,
                                )
                            else:
                                nc.vector.scalar_tensor_tensor(
                                    xs,
                                    po[:, sl],
                                    gr[ti][:, 0:1],
                                    xs,
                                    op0=mybir.AluOpType.mult,
                                    op1=mybir.AluOpType.add,
                                )
                            nc.sync.dma_start(
                                out=o_ap[b, bass.ts(ti, 128), s : s + ln],
                                in_=xs,
                            )
                    yield

            def exhaust(g):
                for _ in g:
                    pass

            # schedule (BPC == 2):
            #   issue b0 then b1 loads (FIFO ring -> loads outrank stores),
            #   emit all of b0 phase 1, then interleave b0's out-phase with
            #   b1's Gram phase, then b1's softmax + out-phase.
            g0, g1 = phase1_gen(0), phase1_gen(1)
            next(g0)  # b0 loads
            next(g1)  # b1 loads
            for _ in range(KP // 2 + 1):
                next(g0)  # b0 k-pair groups + energy drain
            exhaust(g0)  # b0 fills + softmax
            p2 = phase2_gen(0)
            for _ in range(FT):
                next(g1)  # 2 k-pairs of b1 (first slot: all its casts)
                next(p2)  # 1 f-chunk of b0 outs
            exhaust(g1)  # b1 energy drain + fills + softmax
            exhaust(phase2_gen(1))

    nc.compile()
    if not nc.is_finalized():
        nc.finalize()
    return nc


_NC = None


def _get_nc():
    global _NC
    if _NC is None:
        _NC = build()
    return _NC


def _axon_reset():
    """Recover a wedged NeuronCore (NRT_EXEC_UNIT_UNRECOVERABLE) via the
    axon PJRT plugin's reset entry point. Best-effort."""
    try:
        import ctypes

        import jax

        jax.devices()
        lib = ctypes.CDLL("/opt/axon/libaxon_pjrt.so")
        lib.axon_reset.restype = ctypes.c_int64
        return lib.axon_reset() == 0
    except Exception:
        return False


def _run(x, gamma, **kw):
    nc = _get_nc()
    x = np.ascontiguousarray(np.asarray(x, dtype=np.float32).reshape(B, C, N))
    g = np.asarray(gamma, dtype=np.float32).reshape(1)
    in_maps = [
        {"x": x[c * BPC : (c + 1) * BPC], "gamma": g} for c in range(NCORES)
    ]
    try:
        res = run_bass_kernel_spmd(nc, in_maps, list(range(NCORES)), **kw)
    except Exception as e:
        if "unrecoverable" not in str(e).lower():
            raise
        _axon_reset()
        res = run_bass_kernel_spmd(nc, in_maps, list(range(NCORES)), **kw)
    out = np.concatenate([r["out"] for r in res.results], axis=0)
    return out.reshape(B, C, H, W), res


def kernel(x, gamma):
    out, _ = _run(x, gamma)
    return out


# revision 40
# speedup vs baseline: 1.1759x; 1.1759x over previous
"""CAM (channel attention) kernel for Trainium2, SPMD over 8 NeuronCores.

Computation per batch b (reference semantics):
    v      = x[b].reshape(C, N)                      # C=512, N=4096
    energy = v @ v.T                                 # [C, C] Gram over channels
    att    = softmax(max_j(energy) - energy, axis=-1)
           = exp(min_j(energy) - energy) / sum_j(...)   # algebraically identical
    out    = gamma * (att @ v) + x[b]

Distribution: pure data parallel over batch. B=16 -> 2 batches per core.

The kernel sits on the DMA roofline: 16.8MB in + 16.8MB out per core at
~360-400 GB/s ≈ 90us. Matmul work runs in fp8e4 with the DoubleRow perf
mode (256-deep contraction per instruction, 2x rate) so the PE fits
under that roofline; softmax statistics (row min / exp / sum) stay in
fp32 and the residual path is exact fp32.

Per-core pipeline (per batch):
  - x loaded f32 in 6 chunks on the sync HWDGE ring (first chunk 128px
    so the PE starts ~7us in); stores ride the same FIFO ring, which
    auto-prioritizes all loads ahead of all stores
  - each chunk is rounded to fp8 into v8[tp] [128, 2, N] paired layout
    (dim1 = two 128-channel halves, the DoubleRow contraction pairs);
    the converting casts run on whichever engine is idle during that
    batch's Gram phase (DVE for batch 0; ACT/DVE for batch 1, whose
    Gram phase overlaps batch 0's DVE fuse stream)
  - PE transposes v8 128x128 blocks (fp8, step-2 PSUM writes) into
    u8[kp] [128, 2, C] k-pair tiles; ACT evacuates PSUM->SBUF
  - energy: 16 DoubleRow matmuls per m-tile accumulate the Gram in 4
    f32 PSUM banks, upper-triangle blocks only + 5 transpose fills
  - row-softmax: DVE row-min, ACT exp(bias=min, scale=-1) with fused
    f32 row-sum, fp8 att out; DVE reciprocal; gr = gamma / sum
  - att^T via fp8 PE transposes -> at8[tp] [128, 2, C] paired layout
  - out m-tiles: po = sum_tp at8[tp].T @DR v8[tp] per 512-px chunk
  - evacuation fuses scale+residual IN PLACE over the f32 x tiles (one
    DVE op) and streams each piece out; batch b+1's Gram phase is
    interleaved with batch b's out phase so both engines stay fed
"""

import numpy as np

import concourse.bass as bass
import concourse.bacc as bacc
import concourse.tile as tile
from concourse import mybir
from concourse.bass_utils import run_bass_kernel_spmd
from concourse.masks import make_identity

F32 = mybir.dt.float32
FP8 = mybir.dt.float8e4
DR = mybir.MatmulPerfMode.DoubleRow

B, C, H, W = 16, 512, 64, 64
N = H * W                  # 4096
NCORES = 8
BPC = B // NCORES          # batches per core = 2
CT = C // 128              # 4 channel tiles
KT = N // 128              # 32 contraction tiles for the Gram matrix
KP = KT // 2               # 16 DoubleRow contraction pairs
FT = N // 512              # 8 free-dim chunks for the out matmul
TD = 3                     # transpose software-pipeline depth (k-pairs ahead)
# v is loaded as independent SBUF tiles (start, len in pixels). The first is
# tiny so the PE can start transposing early; boundaries are 256-aligned so
# 256-px k-pair transpose reads never cross tiles (except the 128 head, which
# pairs with the start of chunk 1 -- handled by per-128 transpose reads).
CHUNKS = ((0, 128), (128, 384), (512, 512), (1024, 1024), (2048, 1024), (3072, 1024))


def _chunk_of(n0):
    for lc, (s, ln) in enumerate(CHUNKS):
        if s <= n0 < s + ln:
            return lc, s, ln
    raise AssertionError(n0)


def _segments(n0, w):
    """Split [n0, n0+w) at chunk boundaries -> (chunk idx, start, len)."""
    out = []
    while w > 0:
        lc, s, ln = _chunk_of(n0)
        take = min(w, s + ln - n0)
        out.append((lc, n0, take))
        n0 += take
        w -= take
    return out


def build():
    nc = bacc.Bacc(
        "TRN2",
        target_bir_lowering=False,
        debug=False,
        num_devices=NCORES,
    )
    x_d = nc.dram_tensor("x", [BPC, C, N], F32, kind="ExternalInput")
    g_d = nc.dram_tensor("gamma", [1], F32, kind="ExternalInput")
    o_d = nc.dram_tensor("out", [BPC, C, N], F32, kind="ExternalOutput")
    x_ap, g_ap, o_ap = x_d.ap(), g_d.ap(), o_d.ap()

    with tile.TileContext(nc) as tc:
        with (
            tc.tile_pool(name="const", bufs=1) as const_pool,
            tc.tile_pool(name="vb", bufs=2) as v_pool,
            tc.tile_pool(name="v8", bufs=2) as v8_pool,
            tc.tile_pool(name="u8", bufs=8) as u8_pool,
            tc.tile_pool(name="att", bufs=1) as att_pool,
            tc.tile_pool(name="at8", bufs=2) as at8_pool,
            tc.tile_pool(name="stats", bufs=4) as stats_pool,
            tc.tile_pool(name="stg", bufs=5) as stg_pool,
            tc.tile_pool(name="gr", bufs=2) as gr_pool,
            tc.tile_pool(name="epsum", bufs=1, space="PSUM") as e_pool,
            tc.tile_pool(name="tpsum", bufs=2, space="PSUM") as t_pool,
            tc.tile_pool(name="opsum", bufs=2, space="PSUM") as o_pool,
        ):
            ident = const_pool.tile([128, 128], F32)
            make_identity(nc, ident)
            ident8 = const_pool.tile([128, 128], FP8, name="ident8")
            nc.scalar.copy(ident8, ident)

            gam = const_pool.tile([128, 1], F32)
            nc.gpsimd.dma_start(out=gam, in_=g_ap.to_broadcast((128, 1)))

            # per-batch state carried from phase 1 to phase 2
            state = {}

            def vcol(vt, ci, n0, w):
                # slice [128, w] of channel-tile ci at pixel offset n0 out of
                # the chunked v tiles (w must not cross a chunk boundary)
                lc, s, ln = _chunk_of(n0)
                assert n0 + w <= s + ln, (n0, w)
                return vt[lc][:, ci, n0 - s : n0 - s + w]

            def phase1_gen(b):
                vt = [
                    v_pool.tile([128, CT, ln], F32, tag=f"vb{lc}", name=f"vb{lc}")
                    for lc, (s, ln) in enumerate(CHUNKS)
                ]
                xb = x_ap[b].rearrange("(c p) n -> p c n", p=128)
                for lc, (s, ln) in enumerate(CHUNKS):
                    nc.sync.dma_start(out=vt[lc], in_=xb[:, :, s : s + ln])
                # fp8 paired copies: v8[tp][p, h, n] = fp8(v[(2tp+h)*128+p, n])
                v8 = [
                    v8_pool.tile([128, 2, N], FP8, tag=f"v8_{tp}", name=f"v8_{tp}")
                    for tp in range(2)
                ]

                def convert_chunk(lc):
                    # phase-aware engine split: during batch 0's Gram phase
                    # DVE is idle (fast SBUF->SBUF casts), so it takes all of
                    # b0's conversions and ACT keeps a clean u8-evacuation
                    # stream. During batch 1's Gram phase DVE is busy fusing
                    # b0, so b1's conversions go to ACT (small early chunks)
                    # and Pool/DVE (large chunks, only needed late by the
                    # out-matmuls).
                    s, ln = CHUNKS[lc]
                    for tp in range(2):
                        dst = v8[tp][:, :, s : s + ln]
                        src = vt[lc][:, 2 * tp : 2 * tp + 2, :]
                        # always DVE: idle during b0's Gram phase, and for b1
                        # the eager-emitted casts queue ahead of b0's fuse
                        # groups, keeping ACT purely on the evacuation stream
                        nc.vector.tensor_copy(out=dst, in_=src)

                yield  # [L] loads issued

                e = [
                    e_pool.tile([128, C], F32, tag=f"e{m}", name=f"e{m}")
                    for m in range(CT)
                ]

                def energy_mms(kp, u):
                    # full rows (no triangular symmetry): DR rows are cheap
                    # and skipping the lower-triangle fills takes 5 serialized
                    # ACT-copy -> PE-transpose pairs off the softmax critical
                    # path
                    for m in range(CT):
                        nc.tensor.matmul(
                            e[m],
                            u[:, :, bass.ts(m, 128)],
                            u,
                            start=(kp == 0),
                            stop=(kp == KP - 1),
                            perf_mode=DR,
                        )

                converted = 0
                pending = []
                for kp in range(KP):
                    # convert chunks just ahead of the transposes needing them.
                    # For batch 1 convert everything up front: its DVE casts
                    # then queue ahead of most of batch 0's fuse groups (which
                    # have slack -- b0's stores are ring-bound anyway), so
                    # batch 1's Gram phase isn't paced by the fuse stream.
                    while converted < len(CHUNKS) and (
                        b == 1
                        or CHUNKS[converted][0] < min(kp + TD + 1, KP) * 256
                    ):
                        convert_chunk(converted)
                        converted += 1
                    up = t_pool.tile(
                        [128, 2, C, 2], FP8, tag="upsum", name="upsum"
                    )
                    for h in range(2):
                        k = 2 * kp + h
                        for ci in range(CT):
                            nc.tensor.transpose(
                                up[:, h, bass.ts(ci, 128), 0],
                                v8[ci // 2][:, ci % 2, bass.ts(k, 128)],
                                ident8,
                            )
                    u = u8_pool.tile([128, 2, C], FP8, tag="u", name="u")
                    # batch 0 alternates the PSUM evacuation with the
                    # otherwise-idle DVE so two k-pairs drain in parallel;
                    # batch 1 keeps DVE free for batch 0's fuse stream
                    if b == 0 and kp % 2 == 1:
                        nc.vector.tensor_copy(out=u, in_=up[:, :, :, 0])
                    else:
                        nc.scalar.copy(u, up[:, :, :, 0])
                    pending.append((kp, u))
                    if len(pending) > TD:
                        energy_mms(*pending.pop(0))
                    if kp % 2 == 1:
                        yield  # [K] every 2 k-pairs (8 yields)
                while pending:
                    energy_mms(*pending.pop(0))
                yield  # [E] energy fully emitted

                # row softmax (reversed-max form): att = exp(min - e) / sum
                att = []
                gr = []
                for m in range(CT):
                    mn = stats_pool.tile([128, 1], F32, tag="mn", name="mn")
                    nc.vector.tensor_reduce(
                        mn, e[m], axis=mybir.AxisListType.X, op=mybir.AluOpType.min
                    )
                    a = att_pool.tile([128, C], FP8, tag=f"att{m}", name=f"att{m}")
                    s = stats_pool.tile([128, 1], F32, tag="s", name="s")
                    nc.scalar.activation(
                        a,
                        e[m],
                        mybir.ActivationFunctionType.Exp,
                        bias=mn,
                        scale=-1.0,
                        accum_out=s,
                    )
                    r = stats_pool.tile([128, 1], F32, tag="r", name="r")
                    nc.vector.reciprocal(r, s)
                    g = gr_pool.tile([128, 1], F32, tag=f"gr{m}", name=f"gr{m}")
                    nc.vector.tensor_scalar_mul(g, r, gam[:, 0:1])
                    att.append(a)
                    gr.append(g)

                state[b] = (vt, v8, att, gr)

            def phase2_gen(b):
                vt, v8, att, gr = state.pop(b)

                # attT in fp8 paired (DoubleRow) layout:
                # at8[tp][p, h, i] = att[i, (2*tp+h)*128 + p]
                at8 = [
                    at8_pool.tile([128, 2, C], FP8, tag=f"at8_{tp}", name=f"at8_{tp}")
                    for tp in range(2)
                ]
                for tj in range(CT):
                    ap_ps = t_pool.tile(
                        [128, 2, C, 2], FP8, tag="upsum", name="atpsum"
                    )
                    for ti in range(CT):
                        nc.tensor.transpose(
                            ap_ps[:, 0, bass.ts(ti, 128), 0],
                            att[ti][:, bass.ts(tj, 128)],
                            ident8,
                        )
                    nc.scalar.copy(at8[tj // 2][:, tj % 2], ap_ps[:, 0, :, 0])

                for f in range(FT):
                    for ti in range(CT):
                        po = o_pool.tile([128, 512], F32, tag="opsum", name="opsum")
                        for tp in range(2):
                            nc.tensor.matmul(
                                po,
                                at8[tp][:, :, bass.ts(ti, 128)],
                                v8[tp][:, :, bass.ts(f, 512)],
                                start=(tp == 0),
                                stop=(tp == 1),
                                perf_mode=DR,
                            )
                        # final = (po * (gamma/sum_i)) + x, fused IN PLACE over
                        # the x tile, then streamed out. ti==3 takes a 2-op
                        # detour (ACT scale-evacuates PSUM, Pool adds the
                        # residual SBUF-only) so the DVE fuse rate stops
                        # pacing the whole out phase.
                        if ti == 3:
                            tmp = stg_pool.tile([128, 512], F32, tag="p3", name="p3")
                            nc.scalar.activation(
                                tmp,
                                po,
                                mybir.ActivationFunctionType.Copy,
                                scale=gr[ti][:, 0:1],
                            )
                        for lc, s, ln in _segments(f * 512, 512):
                            xs = vcol(vt, ti, s, ln)
                            sl = slice(s - f * 512, s - f * 512 + ln)
                            if ti == 3:
                                nc.gpsimd.tensor_tensor(
                                    out=xs,
                                    in0=tmp[:, sl],
                                    in1=xs,
                                    op=mybir.AluOpType.add,
                                )
                            else:
                                nc.vector.scalar_tensor_tensor(
                                    xs,
                                    po[:, sl],
                                    gr[ti][:, 0:1],
                                    xs,
                                    op0=mybir.AluOpType.mult,
                                    op1=mybir.AluOpType.add,
                                )
                            nc.sync.dma_start(
                                out=o_ap[b, bass.ts(ti, 128), s : s + ln],
                                in_=xs,
                            )
                    yield

            def exhaust(g):
                for _ in g:
                    pass

            # schedule (BPC == 2):
            #   issue b0 then b1 loads (FIFO ring -> loads outrank stores),
            #   emit all of b0 phase 1, then interleave b0's out-phase with
            #   b1's Gram phase, then b1's softmax + out-phase.
            g0, g1 = phase1_gen(0), phase1_gen(1)
            next(g0)  # b0 loads
            next(g1)  # b1 loads
            for _ in range(KP // 2 + 1):
                next(g0)  # b0 k-pair groups + energy drain
            exhaust(g0)  # b0 fills + softmax
            p2 = phase2_gen(0)
            for _ in range(FT):
                next(g1)  # 2 k-pairs of b1 (first slot: all its casts)
                next(p2)  # 1 f-chunk of b0 outs
            exhaust(g1)  # b1 energy drain + fills + softmax
            exhaust(phase2_gen(1))

    nc.compile()
    if not nc.is_finalized():
        nc.finalize()
    return nc


_NC = None


def _get_nc():
    global _NC
    if _NC is None:
        _NC = build()
    return _NC


def _axon_reset():
    """Recover a wedged NeuronCore (NRT_EXEC_UNIT_UNRECOVERABLE) via the
    axon PJRT plugin's reset entry point. Best-effort."""
    try:
        import ctypes

        import jax

        jax.devices()
        lib = ctypes.CDLL("/opt/axon/libaxon_pjrt.so")
        lib.axon_reset.restype = ctypes.c_int64
        return lib.axon_reset() == 0
    except Exception:
        return False


def _run(x, gamma, **kw):
    nc = _get_nc()
    x = np.ascontiguousarray(np.asarray(x, dtype=np.float32).reshape(B, C, N))
    g = np.asarray(gamma, dtype=np.float32).reshape(1)
    in_maps = [
        {"x": x[c * BPC : (c + 1) * BPC], "gamma": g} for c in range(NCORES)
    ]
    try:
        res = run_bass_kernel_spmd(nc, in_maps, list(range(NCORES)), **kw)
    except Exception as e:
        if "unrecoverable" not in str(e).lower():
            raise
        _axon_reset()
        res = run_bass_kernel_spmd(nc, in_maps, list(range(NCORES)), **kw)
    out = np.concatenate([r["out"] for r in res.results], axis=0)
    return out.reshape(B, C, H, W), res


def kernel(x, gamma):
    out, _ = _run(x, gamma)
    return out
